# revision 1
# baseline (speedup 1.0000x reference)
"""Chamfer distance kernel for 8 Trainium2 NeuronCores (Bass/Tile).

Problem: pc1, pc2: [2, 8192, 3] f32.
  dist[b,n,m] = ||pc1[b,n]-pc2[b,m]||^2
  out = mean_n(min_m dist) + mean_m(min_n dist)   (scalar f32)

Strategy:
  * Augmented-matmul: dist[n,m] = L1[:,n] . R2[:,m] with
      L(p) = [-2x,-2y,-2z, |p|^2, 1],  R(p) = [x,y,z, 1, |p|^2]  (K=5)
    so the PE emits complete squared distances straight into PSUM.
  * Each core owns 1/8 of pc1's rows (D-path: dist1 rows are final) and
    1/8 of pc2's rows (E-path, roles swapped: dist2 rows are final).
    Both paths reduce along the free axis only -- no partition reductions,
    no collectives. Host just concatenates the 8 outputs and means.
"""

from contextlib import ExitStack

import numpy as np

import concourse.bass as bass
import concourse.tile as tile
from concourse import bacc, mybir
from concourse.bass_utils import run_bass_kernel_spmd

B = 2
N = 8192  # pc1 points per batch
M = 8192  # pc2 points per batch
NCORES = 8
NLOC = N // NCORES  # 1024 pc1 rows per core (D-path)
MLOC = M // NCORES  # 1024 pc2 rows per core (E-path)

# "f32" (K=5 exact, PE 4 cyc/col) or "bf16" (K=20 hi/lo split, PE 1 cyc/col)
# or "f32r" (K=5, fp32 data in float32r fast-path, PE 1 cyc/col)
MATMUL_MODE = "bf16"

# "plain": DVE tensor_reduce over each full PSUM tile (1 elem/cyc/lane).
# "act":   ACT casts 3/4 of the PSUM tiles to bf16 in SBUF; DVE reduces those
#          at the 2x bf16 rate and direct-reduces the rest from PSUM in fp32.
# (A fused tensor_tensor_reduce with an fp32 PSUM source faults TRN2 HW --
#  NRT_EXEC_UNIT_UNRECOVERABLE -- so that variant is gone.)
REDUCE_MODE = "act3"
BF16 = mybir.dt.bfloat16

PSUM_W = 2048  # psum tile free width (4 banks); 2 bufs = all 8 banks
MM_W = 512  # moving-operand width per matmul
F32 = mybir.dt.float32
FLT_MAX = 3.0e38


def _mode_cfg(mode):
    if mode == "f32":
        return 5, mybir.dt.float32
    if mode == "f32r":
        return 5, mybir.dt.float32r
    if mode == "bf16":
        return 20, mybir.dt.bfloat16
    if mode == "bf15":
        return 15, mybir.dt.bfloat16
    raise ValueError(mode)


def _build_nc(mode=MATMUL_MODE, reps=1, reduce_mode=REDUCE_MODE, mm_w=MM_W):
    K, MM_DT = _mode_cfg(mode)
    nc = bacc.Bacc("TRN2", target_bir_lowering=False, debug=False, num_devices=NCORES)

    al = nc.dram_tensor("al", [B, K, NLOC], MM_DT, kind="ExternalInput")
    br = nc.dram_tensor("br", [B, K, M], MM_DT, kind="ExternalInput")
    bl = nc.dram_tensor("bl", [B, K, MLOC], MM_DT, kind="ExternalInput")
    ar = nc.dram_tensor("ar", [B, K, N], MM_DT, kind="ExternalInput")
    d1 = nc.dram_tensor("d1", [B, NLOC], F32, kind="ExternalOutput")
    d2 = nc.dram_tensor("d2", [B, MLOC], F32, kind="ExternalOutput")

    with tile.TileContext(nc) as tc, ExitStack() as ctx:
        sb = ctx.enter_context(tc.tile_pool(name="sb", bufs=1))
        ps = ctx.enter_context(tc.tile_pool(name="ps", bufs=2, space="PSUM"))
        colp = ctx.enter_context(tc.tile_pool(name="colp", bufs=4))
        scrc = ctx.enter_context(tc.tile_pool(name="scrc", bufs=6))
        scrt = ctx.enter_context(tc.tile_pool(name="scrt", bufs=4))

        def load_inputs():
            # Stage all inputs into SBUF (they are small: <=640KB total).
            lhs_sb = {}  # (path, b) -> [K, 1024] tile
            rhs_sb = {}  # (path, b) -> [K, 8192] tile
            for b in range(B):
                t_al = sb.tile([K, NLOC], MM_DT, name=f"al{b}", tag=f"al{b}")
                nc.sync.dma_start(t_al[:], al.ap()[b])
                lhs_sb[("D", b)] = t_al
                t_br = sb.tile([K, M], MM_DT, name=f"br{b}", tag=f"br{b}")
                nc.sync.dma_start(t_br[:], br.ap()[b])
                rhs_sb[("D", b)] = t_br
                t_bl = sb.tile([K, MLOC], MM_DT, name=f"bl{b}", tag=f"bl{b}")
                nc.sync.dma_start(t_bl[:], bl.ap()[b])
                lhs_sb[("E", b)] = t_bl
                t_ar = sb.tile([K, N], MM_DT, name=f"ar{b}", tag=f"ar{b}")
                nc.sync.dma_start(t_ar[:], ar.ap()[b])
                rhs_sb[("E", b)] = t_ar
            return lhs_sb, rhs_sb

        hoist = reduce_mode.endswith("h")
        base_reduce_mode = reduce_mode[:-1] if hoist else reduce_mode
        hoisted = load_inputs() if hoist else None

        def body():
            lhs_sb, rhs_sb = hoisted if hoist else load_inputs()
            d1cols = sb.tile([128, B * 8], F32, name="d1cols", tag="d1cols")
            d2cols = sb.tile([128, B * 8], F32, name="d2cols", tag="d2cols")

            tile_ctr = [0]

            def row_block(lhsT, rhs, out_col):
                """lhsT [K,128], rhs [K,8192]; writes rowmin [128,1] to out_col."""
                ngrp = rhs.shape[1] // PSUM_W
                mc_dt = BF16 if base_reduce_mode in ("act2", "act3") else F32
                ncols = 2 if base_reduce_mode == "fold" else ngrp
                mc = colp.tile([128, ncols], mc_dt, name="mc", tag="mc")
                folds = []
                for g in range(ngrp):
                    pt = ps.tile([128, PSUM_W], F32, name="pt", tag="pt")
                    for j in range(PSUM_W // mm_w):
                        off = g * PSUM_W + j * mm_w
                        nc.tensor.matmul(
                            pt[:, j * mm_w : (j + 1) * mm_w],
                            lhsT,
                            rhs[:, off : off + mm_w],
                        )
                    # DVE may read at most ONE operand from PSUM per instr.
                    i_tile = tile_ctr[0]
                    tile_ctr[0] += 1
                    if (base_reduce_mode in ("act", "act2") and g % 4 != 0) or (
                        base_reduce_mode == "act3" and (i_tile % 9) % 2 == 0
                    ):
                        sc = scrc.tile([128, PSUM_W], BF16, name="sc", tag="sc")
                        nc.scalar.copy(sc[:], pt[:])
                        nc.vector.tensor_reduce(
                            mc[:, g : g + 1],
                            sc[:],
                            axis=mybir.AxisListType.X,
                            op=mybir.AluOpType.min,
                        )
                    elif base_reduce_mode == "fold" and g != 0:
                        # ACT evacuates PSUM as bf16; DVE folds pairs at the
                        # 2x bf16 tensor_tensor rate, then one 1x reduce.
                        sc = scrc.tile([128, PSUM_W], BF16, name="sc", tag="sc")
                        nc.scalar.copy(sc[:], pt[:])
                        folds.append(sc)
                    elif base_reduce_mode == "nored" and g != ngrp - 1:
                        pass  # probe: skip reduce to expose PE-side time
                    else:
                        nc.vector.tensor_reduce(
                            mc[:, 0:1] if base_reduce_mode == "fold" else mc[:, g : g + 1],
                            pt[:],
                            axis=mybir.AxisListType.X,
                            op=mybir.AluOpType.min,
                        )
                if base_reduce_mode == "fold":
                    while len(folds) > 1:
                        a = folds.pop(0)
                        b = folds.pop(0)
                        f = scrt.tile([128, PSUM_W], BF16, name="ft", tag="ft")
                        nc.vector.tensor_tensor(
                            f[:], a[:], b[:], op=mybir.AluOpType.min
                        )
                        folds.append(f)
                    nc.vector.tensor_reduce(
                        mc[:, 1:2],
                        folds[0][:],
                        axis=mybir.AxisListType.X,
                        op=mybir.AluOpType.min,
                    )
                nc.vector.tensor_reduce(
                    out_col,
                    mc[:],
                    axis=mybir.AxisListType.X,
                    op=mybir.AluOpType.min,
                )

            for b in range(B):
                for t in range(8):
                    row_block(
                        lhs_sb[("D", b)][:, t * 128 : (t + 1) * 128],
                        rhs_sb[("D", b)],
                        d1cols[:, b * 8 + t : b * 8 + t + 1],
                    )
                for t in range(8):
                    row_block(
                        lhs_sb[("E", b)][:, t * 128 : (t + 1) * 128],
                        rhs_sb[("E", b)],
                        d2cols[:, b * 8 + t : b * 8 + t + 1],
                    )

            # DRAM element (b, t*128 + p)  <-  cols[p, b*8 + t]
            nc.sync.dma_start(
                d1.ap().rearrange("b (t p) -> p (b t)", p=128), d1cols[:]
            )
            nc.sync.dma_start(
                d2.ap().rearrange("b (t p) -> p (b t)", p=128), d2cols[:]
            )

        if reps == 1:
            body()
        else:
            with tc.For_i(0, reps, 1):
                body()

    nc.compile()
    return nc


_NC_CACHE = {}


def _get_nc(mode=MATMUL_MODE, reps=1, reduce_mode=REDUCE_MODE, mm_w=MM_W):
    key = (mode, reps, reduce_mode, mm_w)
    if key not in _NC_CACHE:
        _NC_CACHE[key] = _build_nc(mode, reps, reduce_mode, mm_w)
    return _NC_CACHE[key]


def _lform(p):  # [B, n, 3] -> [B, 5, n]
    sq = (p * p).sum(-1)
    one = np.ones_like(sq)
    return np.stack([-2 * p[..., 0], -2 * p[..., 1], -2 * p[..., 2], sq, one], axis=1)


def _rform(p):
    sq = (p * p).sum(-1)
    one = np.ones_like(sq)
    return np.stack([p[..., 0], p[..., 1], p[..., 2], one, sq], axis=1)


def _split_bf16(x):
    import ml_dtypes

    hi = x.astype(ml_dtypes.bfloat16).astype(np.float32)
    lo = (x - hi).astype(ml_dtypes.bfloat16).astype(np.float32)
    return hi, lo


def _pack(x, role, mode=MATMUL_MODE):
    """f32 [B,5,n] -> matmul operand [B,K,n] in MM dtype."""
    if mode in ("f32", "f32r"):
        return np.ascontiguousarray(x, dtype=np.float32)
    import ml_dtypes


    hi, lo = _split_bf16(x)
    if mode == "bf15":
        if role == "l":
            out = np.concatenate([hi, hi, lo], axis=1)
        else:
            out = np.concatenate([hi, lo, hi], axis=1)
    elif role == "l":
        out = np.concatenate([hi, hi, lo, lo], axis=1)
    else:
        out = np.concatenate([hi, lo, hi, lo], axis=1)
    return np.ascontiguousarray(out.astype(ml_dtypes.bfloat16))


def _make_in_maps(pc1, pc2, mode=MATMUL_MODE):
    L1, R1 = _lform(pc1), _rform(pc1)
    L2, R2 = _lform(pc2), _rform(pc2)
    brp = _pack(R2, "r", mode)
    arp = _pack(R1, "r", mode)
    L1p = _pack(L1, "l", mode)
    L2p = _pack(L2, "l", mode)
    in_maps = []
    for c in range(NCORES):
        in_maps.append(
            {
                "al": np.ascontiguousarray(L1p[:, :, c * NLOC : (c + 1) * NLOC]),
                "br": brp,
                "bl": np.ascontiguousarray(L2p[:, :, c * MLOC : (c + 1) * MLOC]),
                "ar": arp,
            }
        )
    return in_maps


def kernel(pc1, pc2):
    pc1 = np.asarray(pc1, dtype=np.float32)
    pc2 = np.asarray(pc2, dtype=np.float32)
    assert pc1.shape == (B, N, 3) and pc2.shape == (B, M, 3)

    in_maps = _make_in_maps(pc1, pc2)
    nc = _get_nc()
    res = run_bass_kernel_spmd(nc, in_maps, list(range(NCORES)))

    d1 = np.concatenate([res.results[c]["d1"] for c in range(NCORES)], axis=1)
    d2 = np.concatenate([res.results[c]["d2"] for c in range(NCORES)], axis=1)
    out = d1.mean(dtype=np.float64) + d2.mean(dtype=np.float64)
    return np.float32(out)

